# revision 3
# baseline (speedup 1.0000x reference)
"""Trainium2 Bass kernel for nn_Attention_11055245820093 (v2).

Swin-style attention block: qkv proj -> per-head scaled dot-product attention
with 2D relative position bias (CLS zero-padded), per-head softplus temperature,
patch-diagonal mask -> proj.

Data-parallel over batch B=64 across 8 NeuronCores (8 batches/core).

v2 design:
  - QKV / V projections in fp8e4 DoubleRow matmuls with 3-term residual
    expansion (x8*whi + x8*wlo + xr*whi), host-side quantization: ~bf16
    accuracy at 0.75x bf16 PE cost and 4x fewer contraction passes.
  - Attention (S, AV, denom) in bf16.
  - Softmax denominators produced pre-broadcast by a ones-weight matmul into
    the same PSUM tile as AV output; one DVE divide produces normalized
    attention output in (c,t) layout (no reciprocal, no broadcast copy).
  - Uniform 128-row tiles: x/K padded with zeros past the token end, junk
    rows killed by zero rows in the multiplicative bias table.
  - One merged exp per (head-pair, batch) over a strided 2-bank PSUM AP.
  - proj in bf16, output copied f32 and DMA'd per tile.
"""

import os
import sys

sys.path.insert(0, "/opt/trn_rl_repo")
os.environ.setdefault("MYCRO_LOCAL_CACHE", "1")

import numpy as np
import ml_dtypes

BF16 = ml_dtypes.bfloat16
E4 = ml_dtypes.float8_e4m3

# Problem constants
B, N, C, H, D = 64, 197, 768, 12, 64
NCORES = 8
BPC = B // NCORES          # 8 batches per core
T = BPC * N                # 1576 tokens per core
TP = T + 64                # padded token cols (for uniform 128-row j-tiles)
NB = 198                   # even batch pitch (dual-fp8 Ldweights needs even offsets)
TK = BPC * NB + 64         # 1648: K-tile region width (even)
XW = 864                   # x a/b part width (multiple of 16 for dual-fp8 ldweights)
XB0 = 4 * NB               # 792: b-part base col
XPW = 1664                 # host-side padded x width (>= XB0 + XW)
TA = 1584                  # attn fp8-pair region pitch (T+8, multiple of 16)
SWP = 2048.0               # fp8 scale for proj weights (descaled at out-copy)
KT = C // 128              # 6 bf16 contraction tiles
KK = C // 256              # 3 fp8 DoubleRow contraction tiles
SCALE = D ** -0.5
SW = 32.0                  # fp8 scale for qkv weights (== SQ)
SWV = 128.0                # fp8 scale for v weights (folded out via wpj)
SQ = 32.0                  # fp8 storage scale for Q/K tiles (folded into exp)

_CACHE = {}

TRACE = False
LAST_RESULTS = None


def _build(finalize=True):
    import concourse.bass as bass
    import concourse.tile as tile
    from concourse import bacc, mybir

    dt = mybir.dt
    f32, bf16, fp8 = dt.float32, dt.bfloat16, dt.float8e4
    fp16 = dt.float16
    AF = mybir.ActivationFunctionType
    OP = mybir.AluOpType
    PM = mybir.MatmulPerfMode
    AP = bass.AP

    nc = bacc.Bacc("TRN2", target_bir_lowering=False, debug=False)

    # DRAM inputs (layouts documented in _host_prep)
    x8a = nc.dram_tensor("x8a", [KK, 128, 2 * XW], fp8, kind="ExternalInput").ap()
    x8b = nc.dram_tensor("x8b", [KK, 128, 2 * XW], fp8, kind="ExternalInput").ap()
    xra = nc.dram_tensor("xra", [KK, 128, 2 * XW], fp8, kind="ExternalInput").ap()
    xrb = nc.dram_tensor("xrb", [KK, 128, 2 * XW], fp8, kind="ExternalInput").ap()
    whi = nc.dram_tensor("whi", [KK, 128, 2 * 1536], fp8, kind="ExternalInput").ap()
    wlo = nc.dram_tensor("wlo", [KK, 128, 2 * 1536], fp8, kind="ExternalInput").ap()
    wvhi = nc.dram_tensor("wvhi", [KK, 128, 2 * C], fp8, kind="ExternalInput").ap()
    wvlo = nc.dram_tensor("wvlo", [KK, 128, 2 * C], fp8, kind="ExternalInput").ap()
    wphi = nc.dram_tensor("wphi", [KK, 128, 2 * 1536], fp8, kind="ExternalInput").ap()
    wplo = nc.dram_tensor("wplo", [KK, 128, 2 * 1536], fp8, kind="ExternalInput").ap()
    bT = nc.dram_tensor("bT", [KT, 128, 788], fp16, kind="ExternalInput").ap()
    bqk = nc.dram_tensor("bqk", [128, 2 * KT], f32, kind="ExternalInput").ap()
    outT = nc.dram_tensor("outT", [KT, 128, T], fp16, kind="ExternalOutput").ap()

    def ap3(t, base, d1_stride, d1_n, d2_stride, d2_n):
        """partition-dim + 2 free dims view of a 2D sbuf tile AP."""
        a = t[:]
        return AP(a.tensor, a.offset + base,
                  [[a.ap[0][0], a.ap[0][1]], [d1_stride, d1_n], [d2_stride, d2_n]])

    def ap3p(t, p0, pn, base, d1_stride, d1_n, d2_stride, d2_n):
        a = t[p0:p0 + pn]
        return AP(a.tensor, a.offset + base,
                  [[a.ap[0][0], pn], [d1_stride, d1_n], [d2_stride, d2_n]])

    with tile.TileContext(nc) as tc:
        from contextlib import ExitStack

        with ExitStack() as ctx:
            cp = ctx.enter_context(tc.tile_pool(name="consts", bufs=1))
            psA = ctx.enter_context(tc.tile_pool(name="psA", bufs=2, space="PSUM"))
            psS = ctx.enter_context(tc.tile_pool(name="psS", bufs=2, space="PSUM"))
            psP = ctx.enter_context(tc.tile_pool(name="psP", bufs=2, space="PSUM"))
            wp = ctx.enter_context(tc.tile_pool(name="work", bufs=2))

            # ---- persistent SBUF tiles; DMAs in consumption order ----
            x8_sb, xr8_sb = [], []
            wvhi_sb, wvlo_sb = [], []
            whi_sb, wlo_sb = [], []
            x8b_sb, xrb_sb = [], []
            for k in range(KK):
                ta = cp.tile([128, 2 * XW], fp8, name=f"x8a{k}", tag=f"x8a{k}")
                nc.sync.dma_start(out=ta[:], in_=x8a[k])
                x8_sb.append(ta)
                tr = cp.tile([128, 2 * XW], fp8, name=f"xra{k}", tag=f"xra{k}")
                nc.scalar.dma_start(out=tr[:], in_=xra[k])
                xr8_sb.append(tr)
            for k in range(KK):
                tb = cp.tile([128, 2 * XW], fp8, name=f"x8b{k}", tag=f"x8b{k}")
                nc.sync.dma_start(out=tb[:], in_=x8b[k])
                x8b_sb.append(tb)
                tr = cp.tile([128, 2 * XW], fp8, name=f"xrb{k}", tag=f"xrb{k}")
                nc.scalar.dma_start(out=tr[:], in_=xrb[k])
                xrb_sb.append(tr)

            def xap(lst_a, lst_b, k, tok, n):
                # x view [128, (2 slots), (n cols @ tok)] across the a/b split
                if tok + n <= XW:
                    return ap3(lst_a[k], tok, XW, 2, 1, n)
                assert tok >= XB0
                return ap3((x8b_sb if lst_a is x8_sb else xrb_sb)[k],
                           tok - XB0, XW, 2, 1, n)
            for k in range(KK):
                t_ = cp.tile([128, 2 * C], fp8, name=f"wvhi{k}", tag=f"wvhi{k}")
                nc.gpsimd.dma_start(out=t_[:], in_=wvhi[k])
                wvhi_sb.append(t_)
            for k in range(KK):
                t_ = cp.tile([128, 2 * C], fp8, name=f"wvlo{k}", tag=f"wvlo{k}")
                nc.gpsimd.dma_start(out=t_[:], in_=wvlo[k])
                wvlo_sb.append(t_)
            for k in range(KK):
                t_ = cp.tile([128, 2 * 1536], fp8, name=f"whi{k}", tag=f"whi{k}")
                whi_sb.append(t_)
            for k in range(KK):
                t_ = cp.tile([128, 2 * 1536], fp8, name=f"wlo{k}", tag=f"wlo{k}")
                wlo_sb.append(t_)
            scratch = cp.tile([1, 4], f32, name="scratch", tag="scratch")
            bqk_sb = cp.tile([128, 2 * KT], f32, name="bqk", tag="bqk")
            nc.gpsimd.dma_start(out=bqk_sb[:], in_=bqk[:])
            bias_sb = [
                cp.tile([128, 788], fp16, name=f"bias{hp}", tag=f"bias{hp}")
                for hp in range(KT)
            ]
            wphi_sb = [
                cp.tile([128, 2 * 1536], fp8, name=f"wphi{k}", tag=f"wphi{k}")
                for k in range(KK)
            ]
            wplo_sb = [
                cp.tile([128, 2 * 1536], fp8, name=f"wplo{k}", tag=f"wplo{k}")
                for k in range(KK)
            ]
            a8_sb = [
                cp.tile([128, 2 * TA], fp8, name=f"a8_{k}", tag=f"a8_{k}")
                for k in range(KK)
            ]
            al_sb = [
                cp.tile([128, 2 * TA], fp8, name=f"al_{k}", tag=f"al_{k}")
                for k in range(KK)
            ]

            # Q tiles [128, T]; K tiles padded [128, TP] (zero tail for junk-row
            # j-tiles of the last batch). fp8, scaled by SQ (descaled in exp).
            q_sb = [cp.tile([128, T], fp8, name=f"q{m}", tag=f"q{m}") for m in range(KT)]
            k_sb = [cp.tile([128, 2 * TK], fp8, name=f"k{m}", tag=f"k{m}") for m in range(KT)]
            for m in range(KT):
                for reg in (0, TK):
                    gap = k_sb[m][:]
                    nc.vector.memset(
                        AP(gap.tensor, gap.offset + reg + 197, [[gap.ap[0][0], 128], [NB, BPC], [1, 1]]),
                        0.0,
                    )
                    nc.vector.memset(k_sb[m][:, reg + BPC * NB: reg + TK], 0.0)

            # V per batch: [128, 2*768] bf16, slot jt in {0,1}: v2[p, jt*768 + c]
            # = SWV * V[token j = jt*128+p, c] (pad rows exactly 0 via x padding)
            v2_sb = [
                cp.tile([128, 2 * C], fp16, name=f"v2_{b}", tag=f"v2_{b}")
                for b in range(BPC)
            ]
            # attention output (c,t), bf16, kk-tile per head-pair
            attn_sb = [
                cp.tile([128, T], fp16, name=f"at{m}", tag=f"at{m}") for m in range(KT)
            ]
            ones_sb = cp.tile([128, 64], fp16, name="ones", tag="ones")
            nc.vector.memset(ones_sb[:], 1.0)

            # ---- Phase B: V in (t, c) slot layout, fp8 DR 3-term ----
            def emit_B(b_range):
                for b in b_range:
                    emit_B_one(b)

            def emit_B_one(b):
                for jt in range(2):
                    tok = b * NB + jt * 128
                    psv = psS.tile([128, 1024], f32, tag="psS")
                    for n2 in range(3):
                        first = True
                        for (xs, ws) in ((x8_sb, wvhi_sb), (x8_sb, wvlo_sb), (xr8_sb, wvhi_sb)):
                            for k in range(KK):
                                nc.tensor.matmul(
                                    psv[:, n2 * 256:(n2 + 1) * 256],
                                    xap(xs, None, k, tok, 128),
                                    ap3(ws[k], n2 * 256, C, 2, 1, 256),
                                    start=first,
                                    stop=(k == KK - 1 and ws is wvhi_sb and xs is xr8_sb),
                                    perf_mode=PM.DoubleRow,
                                )
                                first = False
                    vin = AP(psv[:].tensor, psv[:].offset, [[psv[:].ap[0][0], 128], [1, 768]])
                    if jt == 0:
                        nc.scalar.activation(v2_sb[b][:, 0:C], vin, AF.Copy)
                    else:
                        nc.vector.tensor_copy(v2_sb[b][:, C:2 * C], vin)

            emit_B(range(0, 3))
            # gate the qk-weight DMAs behind phase-B progress so they don't
            # starve B's own input stream on the serial DMA resource
            nc.gpsimd.tensor_copy(scratch[0:1, 0:1], v2_sb[2][0:1, 0:1])
            for k in range(KK):
                nc.gpsimd.dma_start(out=whi_sb[k][:], in_=whi[k])
            for k in range(KK):
                nc.gpsimd.dma_start(out=wlo_sb[k][:], in_=wlo[k])
            emit_B(range(3, BPC))

            # ---- Phase A (by head-pair) interleaved with Phase C ----
            def emit_proj_tile(mt, dest, lo_col=None, terms=3, pitch=N):
                # one psum tile covers 2 batch-chunks of 197 tokens
                for c2 in range(4):
                    ps = psA.tile([128, 394], f32, tag="psA")
                    for half in range(2):
                        tok = (2 * c2 + half) * NB
                        tl = ((x8_sb, whi_sb), (x8_sb, wlo_sb), (xr8_sb, whi_sb))[:terms]
                        first = True
                        for ti, (xs, ws) in enumerate(tl):
                            for k in range(KK):
                                nc.tensor.matmul(
                                    ps[:, half * N:(half + 1) * N],
                                    ap3(ws[k], mt * 128, 1536, 2, 1, 128),
                                    xap(xs, None, k, tok, N),
                                    start=first,
                                    stop=(k == KK - 1 and ti == len(tl) - 1),
                                    perf_mode=PM.DoubleRow,
                                )
                                first = False
                    da = dest[:]
                    dap = AP(da.tensor, da.offset + 2 * c2 * pitch,
                             [[da.ap[0][0], 128], [pitch, 2], [1, N]])
                    nc.scalar.activation(
                        dap,
                        ps[:],
                        AF.Identity,
                        bias=bqk_sb[:, mt:mt + 1],   # host pre-scaled by SQ
                        scale=SQ / SW,               # == 1.0
                    )
                    if lo_col is not None:
                        # residual: lo = (psum + bias) - hi   (fp8 pair)
                        lap = AP(da.tensor, da.offset + lo_col + 2 * c2 * pitch,
                                 [[da.ap[0][0], 128], [pitch, 2], [1, N]])
                        nc.vector.scalar_tensor_tensor(
                            lap,
                            ps[:],
                            bqk_sb[:, mt:mt + 1],
                            dap,
                            mybir.AluOpType.add,
                            mybir.AluOpType.subtract,
                        )

            def emit_A(hp):
                emit_proj_tile(KT + hp, k_sb[hp], lo_col=TK, pitch=NB)  # K^T hi/lo pair
                nc.gpsimd.dma_start(out=bias_sb[hp][:], in_=bT[hp])
                emit_proj_tile(hp, q_sb[hp])             # Q^T tile (fp8 single)

            emit_A(0)
            for hp in range(KT):
                if hp + 1 < KT:
                    emit_A(hp + 1)

                def emit_S(b):
                    # S^T psum: slabs (hh,jt) at cols 0,197,512,709
                    sps = psS.tile([128, 1024], f32, tag="psS")
                    for hh in range(2):
                        for jt in range(2):
                            nc.tensor.matmul(
                                sps[:, hh * 512 + jt * N: hh * 512 + (jt + 1) * N],
                                ap3p(k_sb[hp], 64 * hh, 64,
                                     b * NB + jt * 128, TK, 2, 1, 128),
                                ap3p(q_sb[hp], 64 * hh, 64, b * N, 0, 2, 1, N),
                                start=True,
                                stop=True,
                                perf_mode=PM.DoubleRow,
                            )
                    e = wp.tile([128, 788], fp16, tag="e", bufs=3)
                    sin = AP(sps[:].tensor, sps[:].offset,
                             [[sps[:].ap[0][0], 128], [512, 2], [1, 394]])
                    nc.scalar.activation(e[:], sin, AF.Exp, scale=1.0 / (SQ * SQ))
                    e2 = wp.tile([128, 788], fp16, tag="e2", bufs=3)
                    nc.vector.tensor_mul(e2[:], e[:], bias_sb[hp][:])
                    return e2

                e2_cur = emit_S(0)
                for b in range(BPC):
                    e2_nxt = emit_S(b + 1) if b + 1 < BPC else None
                    e2 = e2_cur
                    # AV + denominator-broadcast into one psum tile
                    pd = psP.tile([128, 512], f32, tag="psP")
                    for hh in range(2):
                        h = 2 * hp + hh
                        for jt in range(2):
                            nc.tensor.matmul(
                                pd[64 * hh:64 * hh + 64, 0:N],
                                v2_sb[b][:, jt * C + h * 64: jt * C + (h + 1) * 64],
                                e2[:, hh * 394 + jt * N: hh * 394 + (jt + 1) * N],
                                start=(jt == 0),
                                stop=(jt == 1),
                            )
                        for jt in range(2):
                            nc.tensor.matmul(
                                pd[64 * hh:64 * hh + 64, 197:394],
                                ones_sb[:],
                                e2[:, hh * 394 + jt * N: hh * 394 + (jt + 1) * N],
                                start=(jt == 0),
                                stop=(jt == 1),
                            )
                    dn = wp.tile([128, N], fp16, tag="dn", bufs=3)
                    with nc.allow_low_precision(
                        reason="softmax denom reciprocal in fp16"
                    ):
                        nc.vector.reciprocal(dn[:], pd[:, 197:394])
                    nc.vector.tensor_mul(
                        attn_sb[hp][:, b * N:(b + 1) * N],
                        pd[:, 0:N],
                        dn[:],
                    )
                    acol = (hp % 2) * TA + b * N
                    nc.gpsimd.tensor_copy(
                        a8_sb[hp // 2][:, acol:acol + N],
                        attn_sb[hp][:, b * N:(b + 1) * N],
                    )
                    nc.gpsimd.tensor_sub(
                        al_sb[hp // 2][:, acol:acol + N],
                        attn_sb[hp][:, b * N:(b + 1) * N],
                        a8_sb[hp // 2][:, acol:acol + N],
                    )
                    e2_cur = e2_nxt

            # ---- Phase D: proj -> out^T(c,t), fp8 DR 3-term; bias on host ----
            for k in range(KK):
                nc.gpsimd.dma_start(out=wphi_sb[k][:], in_=wphi[k])
            for k in range(KK):
                nc.gpsimd.dma_start(out=wplo_sb[k][:], in_=wplo[k])

            def d_rhs(lst, k, col):
                return ap3(lst[k], col, TA, 2, 1, N)

            for mt in range(KT):
                for nt in range(4):
                    last = (mt == KT - 1 and nt == 3)
                    ps = psA.tile([128, 394], f32, tag="psA")
                    for g in range(2):
                        col = nt * 394 + g * 197
                        first = True
                        for (al_, wl_) in ((a8_sb, wphi_sb), (a8_sb, wplo_sb), (al_sb, wphi_sb)):
                            for k in range(KK):
                                nc.tensor.matmul(
                                    ps[:, g * 197:(g + 1) * 197],
                                    ap3(wl_[k], mt * 128, 1536, 2, 1, 128),
                                    d_rhs(al_, k, col),
                                    start=first,
                                    stop=(k == KK - 1 and wl_ is wphi_sb and al_ is al_sb),
                                    perf_mode=PM.DoubleRow,
                                )
                                first = False
                    ot = wp.tile([128, 394], fp16, tag="ot", bufs=3)
                    if last:
                        # split the final copy across both engines to shrink
                        # the end-of-kernel tail
                        nc.vector.tensor_scalar_mul(ot[:, 0:197], ps[:, 0:197], 1.0 / SWP)
                        nc.scalar.activation(ot[:, 197:394], ps[:, 197:394], AF.Copy,
                                             scale=1.0 / SWP)
                        nc.sync.dma_start(
                            out=outT[mt, :, nt * 394:nt * 394 + 197], in_=ot[:, 0:197]
                        )
                        nc.scalar.dma_start(
                            out=outT[mt, :, nt * 394 + 197:(nt + 1) * 394],
                            in_=ot[:, 197:394],
                        )
                    else:
                        if (mt * 4 + nt) % 2 == 0:
                            nc.vector.tensor_scalar_mul(ot[:], ps[:], 1.0 / SWP)
                        else:
                            nc.scalar.activation(ot[:], ps[:], AF.Copy, scale=1.0 / SWP)
                        nc.sync.dma_start(
                            out=outT[mt, :, nt * 394:(nt + 1) * 394], in_=ot[:]
                        )

    if finalize:
        nc.finalize()
    return nc


def _q8(a):
    """fp8 e4m3 hi/lo decomposition of a float32 array."""
    hi = a.astype(E4)
    lo = (a - hi.astype(np.float32)).astype(E4)
    return hi, lo


def _host_prep(x, qkv_w, qkv_b, proj_w, proj_b, rel_table, log_temp, rel_index):
    x = np.asarray(x, np.float32)
    qkv_w = np.asarray(qkv_w, np.float32)
    qkv_b = np.asarray(qkv_b, np.float32)
    proj_w = np.asarray(proj_w, np.float32)
    rel_table = np.asarray(rel_table, np.float32)
    log_temp = np.asarray(log_temp, np.float32)
    rel_index = np.asarray(rel_index)

    temp = np.log1p(np.exp(log_temp.astype(np.float64))).astype(np.float32)
    alpha = (SCALE / temp).astype(np.float32)
    alpha_c = np.repeat(alpha, D)                     # (768,)

    # qk weights: (768_in, 1536_out), q cols scaled by alpha, all scaled by SW
    wqkT = qkv_w[0:2 * C].T.copy()
    wqkT[:, 0:C] *= alpha_c[None, :]
    wqkT *= SW
    # DoubleRow layout: [KK, 128, 2slots, 1536]; c = kk*256 + s*128 + p
    w4 = wqkT.reshape(KK, 2, 128, 2 * C).transpose(0, 2, 1, 3)   # (KK,128,2,1536)
    whi, wlo = _q8(w4)
    whi_np = whi.reshape(KK, 128, 2 * 1536)
    wlo_np = wlo.reshape(KK, 128, 2 * 1536)

    wvT = qkv_w[2 * C:3 * C].T.copy() * SWV
    wv4 = wvT.reshape(KK, 2, 128, C).transpose(0, 2, 1, 3)
    wvhi, wvlo = _q8(wv4)
    wvhi_np = wvhi.reshape(KK, 128, 2 * C)
    wvlo_np = wvlo.reshape(KK, 128, 2 * C)

    wpjT = (proj_w.T * (SWP / SWV)).astype(np.float32)          # (768, 768)
    wp4 = wpjT.reshape(KK, 2, 128, C).transpose(0, 2, 1, 3)     # (KK,128,2,768)
    # stationary cols must span 2*768=1536 per slot region: lhsT slices use
    # [p, s*1536 + m] with m in 0..767 per mt*128 window -> pad to 1536
    wph = np.zeros((KK, 128, 2, 1536), np.float32)
    wph[:, :, :, 0:C] = wp4
    wphi_q, wplo_q = _q8(wph)
    wphi_np = wphi_q.reshape(KK, 128, 2 * 1536)
    wplo_np = wplo_q.reshape(KK, 128, 2 * 1536)

    bq = qkv_b[0:C] * alpha_c
    bk = qkv_b[C:2 * C]
    bqk_np = (SQ * np.concatenate([bq, bk])).reshape(2 * KT, 128).T.copy().astype(np.float32)

    # multiplicative bias table exp(rpb/temp), diag->0, CLS->1, (j,i) transposed,
    # slot layout [hp][p, hh*394 + s*197 + i] = ebT[2hp+hh, s*128+p, i], pad 0
    rpb = rel_table[rel_index]                        # (196,196,H)
    bias = np.zeros((H, N, N), np.float32)
    bias[:, 1:, 1:] = rpb.transpose(2, 0, 1) / temp[:, None, None]
    ebias = np.exp(bias)
    idx = np.arange(1, N)
    ebias[:, idx, idx] = 0.0
    ebT = ebias.transpose(0, 2, 1)                    # (H, j, i)
    bT_np = np.zeros((KT, 128, 788), np.float32)
    for hp in range(KT):
        for hh in range(2):
            for s in range(2):
                rows = 128 if s == 0 else N - 128
                bT_np[hp, 0:rows, hh * 394 + s * 197: hh * 394 + s * 197 + N] = \
                    ebT[2 * hp + hh, s * 128: s * 128 + rows, :]
    bT_np = bT_np.astype(np.float16)

    in_maps = []
    for c in range(NCORES):
        xc = x[c * BPC:(c + 1) * BPC]                  # (BPC, N, C)
        xpad = np.zeros((C, XPW), np.float32)
        for b in range(BPC):
            xpad[:, b * NB:b * NB + N] = xc[b].T
        x4 = xpad.reshape(KK, 2, 128, XPW).transpose(0, 2, 1, 3)  # (KK,128,2,XPW)
        xhi, xlo = _q8(x4)
        in_maps.append(
            {
                "x8a": np.ascontiguousarray(xhi[:, :, :, 0:XW]).reshape(KK, 128, 2 * XW),
                "x8b": np.ascontiguousarray(xhi[:, :, :, XB0:XB0 + XW]).reshape(KK, 128, 2 * XW),
                "xra": np.ascontiguousarray(xlo[:, :, :, 0:XW]).reshape(KK, 128, 2 * XW),
                "xrb": np.ascontiguousarray(xlo[:, :, :, XB0:XB0 + XW]).reshape(KK, 128, 2 * XW),
                "whi": whi_np,
                "wlo": wlo_np,
                "wvhi": wvhi_np,
                "wvlo": wvlo_np,
                "wphi": wphi_np,
                "wplo": wplo_np,
                "bT": bT_np,
                "bqk": bqk_np,
            }
        )
    return in_maps


def _postprocess(res, inputs):
    proj_b = np.asarray(inputs["proj_b"], np.float32)
    proj_w = np.asarray(inputs["proj_w"], np.float32)
    bv = np.asarray(inputs["qkv_b"], np.float32)[2 * C:3 * C]
    b_eff = proj_b + proj_w @ bv
    outs = []
    for c in range(NCORES):
        oT = np.asarray(res.results[c]["outT"]).astype(np.float32).reshape(C, T)
        outs.append(oT.T.reshape(BPC, N, C))
    out = np.concatenate(outs, axis=0) + b_eff[None, None, :]
    return out.astype(np.float32)


def kernel(**inputs) -> np.ndarray:
    global LAST_RESULTS
    from concourse.bass_utils import run_bass_kernel_spmd

    if "nc" not in _CACHE:
        _CACHE["nc"] = _build()
    nc = _CACHE["nc"]

    in_maps = _host_prep(**inputs)
    try:
        res = run_bass_kernel_spmd(
            nc, in_maps, core_ids=list(range(NCORES)), trace=TRACE
        )
    except ModuleNotFoundError:
        res = run_bass_kernel_spmd(
            nc, in_maps, core_ids=list(range(NCORES)), trace=False
        )
    LAST_RESULTS = res
    return _postprocess(res, inputs)
